# revision 4
# baseline (speedup 1.0000x reference)
"""Trainium2 kernel for nn_ColorLoss (retrieval_knn) — v4: dual-drain +
bf16-split matmul + 4-deep PSUM pipelining.  178.5us (v1 baseline: 305.6us).

Same drain architecture as v2 (see kernel_v2.py): K-row matmul writes
e[m,n] = b2[n] - 2 q.g into PSUM; Act copies even 2048-col super-tiles to
SBUF; a custom DVE op (min(Src0,Src1), min-accum) drains odd PSUM tiles
paired with the SBUF copies at 2 columns/cycle; tail adds a2, Relu, Sqrt.

Measured pitfalls this version fixes:
  - fp32 matmuls run at ~2.2 cycles/row on TRN2 (pe_only ablation: 239us)
    — the PE emulates fp32 as hi/lo-bf16 passes.  float32r does not
    compile (BIR verification failure).
  - With 2048-col PSUM tiles (2 bufs), the per-m-tile PE -> Act-copy ->
    DVE-pair dependency chain serializes the engines (~12us/m-tile
    critical path; v2 measured 496us).  v4 uses 1024-col tiles x 4 PSUM
    bufs so the PE runs up to 4 tiles ahead and all engines stream.

v4 replaces each fp32 matmul with an exact-to-~2^-24 bf16 multi-split:
  q_d = qh + qm + ql, g_d = gh + gm + gl (bf16 hi/mid/lo residues),
  b2 = bh + bm + bl.  Keeping products >= 2^-26:
    per d: qh*gh, qh*gm, qm*gh, qh*gl, qm*gm, ql*gh   (6 rows)
    b2:    1*bh, 1*bm, 1*bl                            (3 rows)
  => K = 21 bf16 rows, 1 cycle/row on the PE (4x faster than fp32),
  dropped terms sum to ~2e-7 absolute — far below the fp32 rounding noise
  already present.  MM_MODE="fp32r" instead bitcasts fp32 APs to float32r
  (single-pass, 1 cycle/row) — only safe if the fp32r probe shows
  near-fp32 numerics.
"""

import numpy as np

B, M, N, D = 4, 8192, 8192, 3
N_CORES = 8
MPC = (B * M) // N_CORES  # 4096 queries per core
M_TILES = MPC // 128  # 32
N_SUPER = 1024  # psum super-tile (2 banks; x4 bufs = all of PSUM)
N_GROUPS = N // N_SUPER  # 8
N_PAIRS = N_GROUPS // 2  # 4 fused drain ops per m-tile
N_CHUNK = 512  # one matmul / one psum bank
LOSS_WEIGHT = 1.0
BIG = 3.0e38

MM_MODE = "bf16x3"  # "bf16x3" | "fp32r" | "fp32"
K_BF16 = 21
K_F32 = 4

_CACHE: dict = {}


def _register_minmin_op():
    """Runtime-register the fused min(Src0,Src1) + min-reduce DVE op."""
    import concourse.dve_ops as dops
    from concourse.dve_spec import C2, Spec, Src0, Src1, lower, minn
    from concourse.dve_uop import DveOpSpec

    name = "COLORLOSS_MINMIN_ANT"
    for o in dops.OPS:
        if o.name == name:
            return o

    body = minn(Src0, Src1)

    def _ref(in0, in1, s0, s1, imm2):
        b = np.minimum(in0, in1).astype(np.float32)
        acc = np.minimum(
            np.float32(imm2), b.reshape(b.shape[0], -1).min(axis=-1, keepdims=True)
        ).astype(np.float32)
        return b, acc

    spec = Spec(body=body, accum=minn, accum_init=C2, reference=_ref)
    row = dops._CUSTOM_DVE_ROW_BASE + len(dops.OPS)
    assert row < 0x20, "custom DVE row overflow"
    shas = {}
    for ver in ("v3", "v4"):
        s = DveOpSpec(name=name, opcode=row, uops=lower(spec, ver=ver), rd1_en=True)
        shas[ver] = s.sha(ver)
    op = dops.DveOp(name, spec, subdim=False, uops_sha=shas)
    dops.OPS.append(op)
    dops._SUB_OPCODE_FOR_NAME[name] = row
    return op


def _build_module(reps: int | None = None, ablation: str = "full",
                  mm_mode: str | None = None):
    from contextlib import ExitStack

    import concourse.mybir as mybir
    import concourse.tile as tile
    from concourse import bacc

    mm_mode = mm_mode or MM_MODE
    minmin_op = _register_minmin_op()

    nc = bacc.Bacc(
        "TRN2", target_bir_lowering=False, debug=False, num_devices=N_CORES
    )
    f32 = mybir.dt.float32
    bf16 = mybir.dt.bfloat16
    KK = K_BF16 if mm_mode == "bf16x3" else K_F32
    mm_dt = bf16 if mm_mode == "bf16x3" else f32
    qT_d = nc.dram_tensor("qT", [KK, MPC], mm_dt, kind="ExternalInput").ap()
    gT_d = nc.dram_tensor("gT", [KK, N], mm_dt, kind="ExternalInput").ap()
    a2t_d = nc.dram_tensor("a2t", [128, M_TILES], f32, kind="ExternalInput").ap()
    mind_d = nc.dram_tensor("mind", [128, M_TILES], f32, kind="ExternalOutput").ap()

    with tile.TileContext(nc) as tc:
        with ExitStack() as ctx:
            inp = ctx.enter_context(tc.tile_pool(name="inp", bufs=1))
            psum = ctx.enter_context(tc.tile_pool(name="ps", bufs=4, space="PSUM"))
            scopy = ctx.enter_context(tc.tile_pool(name="sc", bufs=4))
            small = ctx.enter_context(tc.tile_pool(name="sm", bufs=4))
            accp = ctx.enter_context(tc.tile_pool(name="acc", bufs=1))

            # q/g replicated at partition bases {0,32,64,96}: each n-chunk's
            # matmul runs in its own 32-row group.
            q_sb = inp.tile([128, MPC], mm_dt)
            g_sb = inp.tile([128, N], mm_dt)
            for i in range(4):
                nc.sync.dma_start(q_sb[32 * i : 32 * i + KK, :], qT_d[:])
                nc.sync.dma_start(g_sb[32 * i : 32 * i + KK, :], gT_d[:])
            a2_sb = inp.tile([128, M_TILES], f32)
            nc.sync.dma_start(a2_sb[:], a2t_d[:])

            acc = accp.tile([128, M_TILES], f32)

            def mm(out_ap, lhsT, rhs, tile_position):
                if mm_mode == "fp32r":
                    r32 = mybir.dt.float32r
                    lhsT = lhsT.bitcast(r32)
                    rhs = rhs.bitcast(r32)
                nc.tensor.matmul(
                    out_ap, lhsT, rhs, start=True, stop=True,
                    tile_position=tile_position,
                )

            def body():
                _emit_body(nc, tc, mybir, minmin_op, mm, KK, q_sb, g_sb, a2_sb,
                           acc, psum, scopy, small, ablation)

            if reps is None:
                body()
            else:
                with tc.For_i(0, reps, 1):
                    body()

            nc.sync.dma_start(mind_d[:], acc[:])

    nc.compile()
    return nc


def _emit_body(nc, tc, mybir, minmin_op, mm, KK, q_sb, g_sb, a2_sb, acc, psum,
               scopy, small, ablation="full"):
    f32 = mybir.dt.float32
    mins_all = small.tile([128, M_TILES * N_PAIRS], f32, tag="mins_all")
    for mi in range(M_TILES):
        s_prev = None
        for g in range(N_GROUPS):
            pt_t = psum.tile([128, N_SUPER], f32, tag="pt")
            pt = pt_t[:]
            if ablation != "drain_only":
                for i in range(N_SUPER // N_CHUNK):  # n-chunks of 512
                    n0 = g * N_SUPER + i * N_CHUNK
                    band = (g * (N_SUPER // N_CHUNK) + i) % 4
                    mm(
                        pt[:, i * N_CHUNK : (i + 1) * N_CHUNK],
                        q_sb[32 * band : 32 * band + KK,
                             mi * 128 : (mi + 1) * 128],
                        g_sb[32 * band : 32 * band + KK, n0 : n0 + N_CHUNK],
                        (32 * band, 0),
                    )
            else:
                for i in range(4):  # touch psum cheaply: registered writer
                    mm(
                        pt[:, i * N_CHUNK : i * N_CHUNK + 16],
                        q_sb[0:KK, mi * 128 : mi * 128 + 128],
                        g_sb[0:KK, 0:16],
                        None,
                    )
            if ablation == "pe_only":
                continue
            if g % 2 == 0:
                s_t = scopy.tile([128, N_SUPER], f32, tag="scopy")
                nc.scalar.copy(s_t[:], pt)
                s_prev = s_t
            else:
                pair = mi * N_PAIRS + g // 2
                nc.vector._custom_dve(
                    minmin_op,
                    out=pt,  # in-place over the psum tile: no SBUF write
                    in0=pt,
                    in1=s_prev[:],
                    imm2=BIG,
                    accum_out=mins_all[:, pair : pair + 1],
                )
    if ablation == "pe_only":
        nc.gpsimd.memset(acc[:], 0.0)
        return
    # min over pairs; d2min = min_e + a2; clamp; sqrt
    dmin = small.tile([128, M_TILES], f32, tag="dmin")
    nc.vector.tensor_reduce(
        dmin[:],
        mins_all[:].rearrange("p (m g) -> p m g", g=N_PAIRS),
        axis=mybir.AxisListType.X,
        op=mybir.AluOpType.min,
    )
    d2 = small.tile([128, M_TILES], f32, tag="d2")
    nc.vector.tensor_tensor(d2[:], dmin[:], a2_sb[:], op=mybir.AluOpType.add)
    dclamp = small.tile([128, M_TILES], f32, tag="dclamp")
    nc.scalar.activation(dclamp[:], d2[:], mybir.ActivationFunctionType.Relu)
    nc.scalar.activation(acc[:], dclamp[:], mybir.ActivationFunctionType.Sqrt)


def _split3(x: np.ndarray):
    """x (fp32) -> (hi, mid, lo) bf16 arrays with hi+mid+lo ~= x (~2^-25)."""
    import ml_dtypes

    bf = ml_dtypes.bfloat16
    hi = x.astype(bf)
    r1 = (x - hi.astype(np.float32)).astype(np.float32)
    mid = r1.astype(bf)
    r2 = (r1 - mid.astype(np.float32)).astype(np.float32)
    lo = r2.astype(bf)
    return hi, mid, lo


def _prep_in_maps(pred_colors: np.ndarray, gt_colors: np.ndarray):
    import ml_dtypes

    bf = ml_dtypes.bfloat16
    pred_colors = np.asarray(pred_colors, dtype=np.float32)
    gt_colors = np.asarray(gt_colors, dtype=np.float32)
    in_maps = []
    for c in range(N_CORES):
        b, h = divmod(c, N_CORES // B)
        q = pred_colors[b, h * MPC : (h + 1) * MPC]  # [MPC, 3]
        g = gt_colors[b]  # [N, 3]
        a2 = (q * q).sum(axis=-1, dtype=np.float32)
        b2 = (g * g).sum(axis=-1, dtype=np.float32)
        if MM_MODE == "bf16x3":
            qh, qm, ql = _split3(q.T)  # [3, MPC] each
            g2 = -2.0 * g.T  # [3, N]
            # decompose g first, then scale by -2 (exact power-of-two)
            gh, gm, gl = _split3(g.T)
            gh = (-2.0 * gh.astype(np.float32)).astype(bf)
            gm = (-2.0 * gm.astype(np.float32)).astype(bf)
            gl = (-2.0 * gl.astype(np.float32)).astype(bf)
            bh, bm, bl = _split3(b2[None, :])
            ones = np.ones((1, MPC), bf)
            qT = np.concatenate(
                [qh, qh, qm, qh, qm, ql, ones, ones, ones], axis=0
            )  # [21, MPC]
            gT = np.concatenate([gh, gm, gh, gl, gm, gh, bh, bm, bl], axis=0)
            assert qT.shape == (K_BF16, MPC) and gT.shape == (K_BF16, N)
        else:
            qT = np.empty((K_F32, MPC), dtype=np.float32)
            qT[:3] = q.T
            qT[3] = 1.0
            gT = np.empty((K_F32, N), dtype=np.float32)
            gT[:3] = -2.0 * g.T
            gT[3] = b2
        in_maps.append(
            {
                "qT": np.ascontiguousarray(qT),
                "gT": np.ascontiguousarray(gT),
                "a2t": np.ascontiguousarray(a2.reshape(M_TILES, 128).T),
            }
        )
    return in_maps


def _get_module(reps: int | None = None):
    key = ("nc", reps, MM_MODE)
    if key not in _CACHE:
        _CACHE[key] = _build_module(reps)
    return _CACHE[key]


def kernel(pred_colors: np.ndarray, gt_colors: np.ndarray) -> np.ndarray:
    import time

    from concourse.bass_utils import run_bass_kernel_spmd

    nc = _get_module()
    in_maps = _prep_in_maps(pred_colors, gt_colors)
    last_err = None
    for attempt in range(3):  # first call after an unclean prior process can
        try:                  # hit a transient "device unrecoverable"; retry
            res = run_bass_kernel_spmd(nc, in_maps, core_ids=list(range(N_CORES)))
            break
        except Exception as e:  # noqa: BLE001
            last_err = e
            time.sleep(2.0)
            try:  # a fresh PJRT client clears terminal-side device state
                import jax

                jax.clear_backends()
            except Exception:  # noqa: BLE001
                pass
    else:
        raise last_err
    mins = np.stack([res.results[c]["mind"] for c in range(N_CORES)])
    out = np.mean(mins, dtype=np.float64) * LOSS_WEIGHT
    return np.asarray(out, dtype=np.float32)
